# revision 5
# baseline (speedup 1.0000x reference)
"""AttGNN kernel for 8 Trainium2 NeuronCores (Bass/Tile).

Math (reference):
    sup2 = sup + I
    h    = feat @ W_map                      [N, 64]
    s    = h @ U ; t = h @ V                 [N, 1]
    att  = softmax_rows(mask(tanh(s_i + t_j + b), sup2[j, i] > 0))   [N, N]
    gat  = tanh(att @ h)                     [N, 64]
    out  = normalize_rows(relu((sup2 @ gat) @ W_gcn))                [N, 64]

Distribution: 1D row-shard of sup/att over 8 cores (1024 rows each).
Both the attention mask and the two big matmuls need sup2 with the
*global* node index on SBUF partitions, i.e. the transpose of the shard
(T[c, j'] = sup2[shard j', c]).  The per-core input buffer is marshalled
host-side in that layout, pre-cast to bf16 (halves HBM traffic and
keeps the load on HWDGE instead of a gpsimd cast stream).

Softmax trick: softmax is scale-invariant, so exp(tanh(z)) can be
replaced by any g(z) with log g(z) = tanh(z) + const to within the
error budget.  g(z) = sigmoid(A z + B) + D with (A, B, D) fit by
minimax in log space matches within +/-0.32%, turning two full ACT
passes (tanh, exp) over the N x S attention block into one sigmoid
pass.  A is folded into U, V host-side; B (+ A*b_map) rides the
per-partition activation bias.

Mask + D in ONE DVE op: supT is pre-scaled by 2**100 host-side (exact,
power of two; 2**-100 is folded into W_gcn), so every nonzero entry of
T is >= 6e24 while 0 stays 0.  Then
    n = min(sigmoid(t' + s'_c) + D, T)     # scalar_tensor_tensor, 2x
equals mask * (sigmoid + D) exactly: 0 < sig + D < 1.16 << T_nonzero.
No separate mask tile, and no ucorr correction matmul stream.

Per core (c = global node index, 64 tiles of 128; j' = local shard row):
  phase 1:  unnorm[65, j'] = sum_c [h | 1][c, :]^T n[c, j']   (PE, PSUM acc)
            row 64 is the softmax denominator d[j'].
            gat[j', :] = tanh(unnorm[0:64, j'] / d[j'])
  all-gather gat (bf16) -> full [8192, 64]
  phase 2:  M[d, i'] = sum_j gat[j, d] T[j, i']               (PE)
            pre[e, i'] = W_gcn^T M                            (PE)
            out[i', :] = normalize(relu(pre))^T               -> store
"""

import os
import numpy as np

N = 8192
DIN = 128
DG = 64
M_CORES = 8
S = N // M_CORES          # 1024 shard rows per core
P = 128                   # partitions
NCT = N // P              # 64 c-tiles
NPAIR = NCT // 2          # 32 pairs of c-tiles
F2 = 2 * S                # 2048 free elems per pair tile

# minimax fit of log(sigmoid(A z + B) + D) ~ tanh(z) + const  (z in [-13, 13])
SIG_A = 2.14235191
SIG_B = -0.99688723
SIG_D = 0.15764918

_built = {}


def _build(reps=1):
    skip_tail = bool(int(os.environ.get("K_SKIP_TAIL", "0")))
    skip_p2 = bool(int(os.environ.get("K_SKIP_P2", "0")))
    skip_main = bool(int(os.environ.get("K_SKIP_MAIN", "0")))
    pool_tt = int(os.environ.get("K_POOL_TT", "8"))
    skip_ag = bool(int(os.environ.get("K_SKIP_AG", "0")))
    import concourse.bass as bass
    import concourse.bacc as bacc
    import concourse.mybir as mybir
    import concourse.tile as tile
    from concourse.masks import make_identity

    f32 = mybir.dt.float32
    bf16 = mybir.dt.bfloat16
    fp8 = mybir.dt.float8e4
    Alu = mybir.AluOpType
    Act = mybir.ActivationFunctionType

    nc = bacc.Bacc(None)

    supT = nc.declare_dram_parameter("supT", [N, S], bf16, isOutput=False)
    featT = nc.declare_dram_parameter("featT", [DIN, N], bf16, isOutput=False)
    featTs = nc.declare_dram_parameter("featTs", [DIN, S], bf16, isOutput=False)
    W_map = nc.declare_dram_parameter("W_map", [DIN, DG], bf16, isOutput=False)
    # wu = W_map @ (A U), wv = W_map @ (A V): s' = featT^T wu, t' = featTs^T wv
    wu_in = nc.declare_dram_parameter("wu", [DIN, 1], bf16, isOutput=False)
    wv_in = nc.declare_dram_parameter("wv", [DIN, 1], bf16, isOutput=False)
    bfit_in = nc.declare_dram_parameter("bfit", [1], f32, isOutput=False)
    W_gcn = nc.declare_dram_parameter("W_gcn", [DG, DG], f32, isOutput=False)
    out_sh = nc.declare_dram_parameter("out_shard", [S, DG], f32, isOutput=True)

    gat_in = nc.dram_tensor("gat_in", [S * DG], fp8)
    gat_all = nc.dram_tensor("gat_all", [M_CORES * S * DG], fp8, addr_space="Shared")

    with tile.TileContext(nc) as tc:
        with (
            tc.tile_pool(name="stat", bufs=1) as stat,
            tc.tile_pool(name="setup", bufs=2) as setup,
            tc.tile_pool(name="tpool", bufs=NPAIR) as tpool,
            tc.tile_pool(name="ring", bufs=2) as ring,
            tc.tile_pool(name="tail8", bufs=8) as tail8,
            tc.tile_pool(name="ps_acc", bufs=1, space="PSUM") as ps_acc,
            tc.tile_pool(name="psx", bufs=2, space="PSUM") as psx,
        ):
            for _rep in range(reps):
                # ---------------- constants ----------------
                ident_f = stat.tile([P, P], f32, tag="ident_f")
                make_identity(nc, ident_f[:])
                ident_b = stat.tile([P, P], bf16, tag="ident_b")
                make_identity(nc, ident_b[:])
                ones_row = stat.tile([1, P], f32, tag="ones_row")
                nc.gpsimd.memset(ones_row[:], 1.0)
                bigH = stat.tile([P, NCT * (DG + 1)], bf16, tag="bigH")
                nc.gpsimd.memset(
                    bigH[:].rearrange("p (ct w) -> p ct w", w=DG + 1)[:, :, DG : DG + 1],
                    1.0,
                )

                wmap_sb = stat.tile([DIN, DG], bf16, tag="wmap")
                nc.sync.dma_start(wmap_sb[:], W_map[:])
                wu_sb = stat.tile([DIN, 1], bf16, tag="wu")
                nc.sync.dma_start(wu_sb[:], wu_in[:])
                wv_sb = stat.tile([DIN, 1], bf16, tag="wv")
                nc.sync.dma_start(wv_sb[:], wv_in[:])
                b_sb = stat.tile([1, 1], f32, tag="b")
                nc.sync.dma_start(b_sb[:], bfit_in[:])
                wgcn_sb = stat.tile([DG, DG], f32, tag="wgcn")
                nc.sync.dma_start(wgcn_sb[:], W_gcn[:])
                # broadcast bfit to [P, 1] for per-partition activation bias
                ps_bb = psx.tile([P, 1], f32, tag="ps")
                nc.tensor.matmul(ps_bb[:], ones_row[:], b_sb[:], start=True, stop=True)
                b_bc = stat.tile([P, 1], f32, tag="b_bc")
                nc.vector.tensor_copy(b_bc[:], ps_bb[:])

                # ---------------- setup input loads (ahead of the T stream,
                # they gate t_bc / s / bigH and are small) ------------------
                fsh = setup.tile([DIN, S], bf16, tag="fch")
                nc.sync.dma_start(fsh[:], featTs[:])
                f_chunks = []
                for g in range(8):
                    fch = setup.tile([DIN, S], bf16, tag=f"fg{g}", bufs=1)
                    f_chunks.append(fch)
                    nc.sync.dma_start(fch[:], featT[:, g * S : (g + 1) * S])

                # ---------------- T load stream ----------------------------
                t_tiles = []
                for p in range(NPAIR):
                    tp = tpool.tile([P, F2], bf16, tag="T")
                    t_tiles.append(tp)
                    for half in range(2):
                        nc.sync.dma_start(
                            tp[:, half * S : (half + 1) * S],
                            supT[p * 256 + half * P : p * 256 + (half + 1) * P, :],
                        )

                # ---------------- setup: t first, then s / bigH ----------------
                # t'_bc[p, j] = (W_map V')^T featTs = wv^T featTs, already
                # broadcast to all 128 partitions: lhsT = wv replicated.
                wvB = stat.tile([DIN, P], bf16, tag="wvB")
                nc.vector.tensor_copy(wvB[:], wv_sb[:].broadcast_to([DIN, P]))
                t_bc = stat.tile([P, S], bf16, tag="t_bc")
                for half in range(2):
                    ps_b = psx.tile([P, 512], f32, tag="ps")
                    nc.tensor.matmul(
                        ps_b[:], wvB[:], fsh[:, half * 512 : (half + 1) * 512],
                        start=True, stop=True,
                    )
                    nc.vector.tensor_copy(t_bc[:, half * 512 : (half + 1) * 512], ps_b[:])

                # s' and h-tiles, chunk by chunk; s' in 8 small tiles so the
                # main loop can start as soon as the first chunk is done.
                sb_tiles = []
                for g in range(8):
                    fch = f_chunks[g]
                    # s' column per c-tile: featT_chunk^T @ wu
                    ps_s = psx.tile([P, 8], f32, tag="ps")
                    for k in range(8):
                        nc.tensor.matmul(
                            ps_s[:, k : k + 1],
                            fch[:, k * P : (k + 1) * P],
                            wu_sb[:],
                            start=True,
                            stop=True,
                        )
                    # s_b = s' + (A*b_map + B): per-partition sigmoid bias
                    s_g = stat.tile([P, 8], f32, tag=f"s_{g}")
                    sb_tiles.append(s_g)
                    nc.scalar.activation(
                        s_g[:], ps_s[:], Act.Identity, bias=b_bc[:], scale=1.0
                    )
                    # bigH h-chunks: batch the 8 PSUM->SBUF copies into one
                    ps_bh = psx.tile([P, 8 * DG], f32, tag="ps")
                    for k in range(8):
                        nc.tensor.matmul(
                            ps_bh[:, k * DG : (k + 1) * DG],
                            fch[:, k * P : (k + 1) * P],
                            wmap_sb[:],
                            start=True,
                            stop=True,
                        )
                    nc.vector.tensor_copy(
                        bigH[:, g * 8 * (DG + 1) : (g + 1) * 8 * (DG + 1)]
                        .rearrange("p (ct w) -> p ct w", w=DG + 1)[:, :, 0:DG],
                        ps_bh[:].rearrange("p (ct w) -> p ct w", w=DG),
                    )

                # PSUM accumulators (1 bank each)
                un0 = ps_acc.tile([DG + 1, 512], f32, tag="un0")
                un1 = ps_acc.tile([DG + 1, 512], f32, tag="un1")
                unnorm = (un0, un1)
                m0 = ps_acc.tile([DG, 512], f32, tag="m0")
                m1 = ps_acc.tile([DG, 512], f32, tag="m1")
                mm = (m0, m1)

                # ---------------- phase 1 main loop ----------------
                for p in ([] if skip_main else range(NPAIR)):
                    tp = t_tiles[p]
                    g = ring.tile([P, F2], bf16, tag="g")
                    for half in range(2):
                        ct = 2 * p + half
                        nc.scalar.activation(
                            g[:, half * S : (half + 1) * S], t_bc[:],
                            Act.Sigmoid, bias=sb_tiles[ct // 8][:, ct % 8 : ct % 8 + 1],
                            scale=1.0,
                        )
                    n = ring.tile([P, F2], bf16, tag="n")
                    nc.vector.scalar_tensor_tensor(
                        n[:], g[:], SIG_D, tp[:], Alu.add, Alu.min
                    )
                    for half in range(2):
                        ct = 2 * p + half
                        lhs = bigH[:, ct * (DG + 1) : (ct + 1) * (DG + 1)]
                        for jb in range(2):
                            sl = slice(half * S + jb * 512, half * S + (jb + 1) * 512)
                            nc.tensor.matmul(
                                unnorm[jb][:], lhs, n[:, sl],
                                start=(p == 0 and half == 0),
                                stop=(p == NPAIR - 1 and half == 1),
                            )

                if skip_tail:
                    zz = tail8.tile([P, DG], f32, tag='fin')
                    nc.vector.memset(zz[:], 0.0)
                    for q in range(8):
                        nc.sync.dma_start(out_sh[q * P : (q + 1) * P, :], zz[:])
                else:
                    # ---------------- tail: gat, all-gather -----------------------
                    d_sb = stat.tile([1, S], f32, tag="d_sb")
                    for jb in range(2):
                        nc.scalar.activation(
                            d_sb[:, jb * 512 : (jb + 1) * 512],
                            unnorm[jb][DG : DG + 1, :], Act.Copy,
                        )
                    rec = ring.tile([DG, S], f32, tag="n")
                    for jb in range(2):
                        ps_d = psx.tile([DG, 512], f32, tag="ps")
                        nc.tensor.matmul(
                            ps_d[:], ones_row[:, 0:DG],
                            d_sb[:, jb * 512 : (jb + 1) * 512],
                            start=True, stop=True,
                        )
                        nc.vector.reciprocal(rec[:, jb * 512 : (jb + 1) * 512], ps_d[:])
                    gv = ring.tile([DG, S], f32, tag="mc")
                    for jb in range(2):
                        nc.vector.tensor_mul(
                            gv[:, jb * 512 : (jb + 1) * 512],
                            unnorm[jb][0:DG, :],
                            rec[:, jb * 512 : (jb + 1) * 512],
                        )
                    gatT = stat.tile([DG, S], bf16, tag="gatT")
                    nc.scalar.activation(gatT[:], gv[:], Act.Tanh)
                    # transpose gatT -> gat natural [1024, 64]; one batched
                    # store, ONE collective (fixed cost dominates), two
                    # ct-major reloads.
                    gn = stat.tile([P, 8 * DG], fp8, tag="gn")
                    for q in range(8):
                        ps_g = psx.tile([P, DG], bf16, tag="ps")
                        nc.tensor.transpose(
                            ps_g[:], gatT[:, q * P : (q + 1) * P],
                            ident_b[0:DG, 0:DG],
                        )
                        nc.vector.tensor_copy(
                            gn[:, q * DG : (q + 1) * DG], ps_g[:]
                        )
                    nc.sync.dma_start(
                        gat_in[:].rearrange("(q p d) -> p q d", q=8, p=P),
                        gn[:].rearrange("p (q d) -> p q d", d=DG),
                    )
                    if not skip_ag:
                        nc.gpsimd.collective_compute(
                            "AllGather",
                            Alu.bypass,
                            replica_groups=[list(range(M_CORES))],
                            ins=[gat_in[:]],
                            outs=[gat_all[:]],
                        )
                    else:
                        nc.sync.dma_start(gat_all[0 : S * DG], gat_in[:])
                    gat_sb = []
                    for hh in range(2):
                        # reuse a dead featT-chunk buffer (same byte size)
                        gsb8 = setup.tile([P, 32 * DG], fp8, tag=f"fg{hh}", bufs=1)
                        nc.sync.dma_start(
                            gsb8[:].rearrange("p (ct d) -> p ct d", d=DG),
                            gat_all[
                                hh * 32 * P * DG : (hh + 1) * 32 * P * DG
                            ].rearrange("(ct p d) -> p ct d", p=P, d=DG),
                        )
                        gsb = setup.tile([P, 32 * DG], bf16, tag="fch")
                        gat_sb.append(gsb)
                        nc.vector.tensor_copy(gsb[:], gsb8[:])

                    if skip_p2:
                        zz2 = tail8.tile([P, DG], f32, tag='fin')
                        nc.vector.memset(zz2[:], 0.0)
                        for q in range(8):
                            nc.sync.dma_start(out_sh[q * P : (q + 1) * P, :], zz2[:])
                    else:
                        # ---------------- phase 2 (jb-outer: jb=0's post-chain
                        # overlaps jb=1's matmul sweep) ----------------
                        m_sb = ring.tile([DG, S], f32, tag="mc")
                        reluT = ring.tile([DG, S], f32, tag="n")
                        onats = [None] * 8
                        n2_all = stat.tile([P, 8], f32, tag="n2_all")
                        sqs = stat.tile([P, DG], f32, tag="sqs")
                        for jb in range(2):
                            first = True
                            for hh in range(2):
                                for rank in range(8):
                                    for k in range(4):
                                        ct = hh * 32 + rank * 4 + k
                                        p_idx, half = ct // 2, ct % 2
                                        lhs = gat_sb[hh][
                                            :, (rank * 4 + k) * DG : (rank * 4 + k + 1) * DG
                                        ]
                                        nc.tensor.matmul(
                                            mm[jb][:],
                                            lhs,
                                            t_tiles[p_idx][
                                                :,
                                                half * S + jb * 512 : half * S
                                                + (jb + 1) * 512,
                                            ],
                                            start=first,
                                            stop=(hh == 1 and rank == 7 and k == 3),
                                        )
                                        first = False
                            nc.vector.tensor_copy(
                                m_sb[:, jb * 512 : (jb + 1) * 512], mm[jb][:]
                            )
                            ps_o = psx.tile([DG, 512], f32, tag="ps")
                            nc.tensor.matmul(
                                ps_o[:], wgcn_sb[:], m_sb[:, jb * 512 : (jb + 1) * 512],
                                start=True, stop=True,
                            )
                            nc.scalar.activation(
                                reluT[:, jb * 512 : (jb + 1) * 512], ps_o[:], Act.Relu
                            )
                            for q in range(jb * 4, jb * 4 + 4):
                                ps_t2 = psx.tile([P, DG], f32, tag="ps")
                                nc.tensor.transpose(
                                    ps_t2[:], reluT[:, q * P : (q + 1) * P],
                                    ident_f[0:DG, 0:DG],
                                )
                                onat = tail8.tile([P, DG], f32, tag="onat")
                                nc.vector.tensor_copy(onat[:], ps_t2[:])
                                onats[q] = onat
                                nc.scalar.activation(
                                    sqs[:], ps_t2[:], Act.Square,
                                    accum_out=n2_all[:, q : q + 1],
                                )

                        # ---------------- normalize + store ----------------
                        nrm = stat.tile([P, 8], f32, tag="nrm")
                        nc.scalar.activation(nrm[:], n2_all[:], Act.Sqrt)
                        nc.vector.tensor_scalar_max(nrm[:], nrm[:], 1e-12)
                        rcl = stat.tile([P, 8], f32, tag="rcl")
                        nc.vector.reciprocal(rcl[:], nrm[:])
                        for q in range(8):
                            fin = tail8.tile([P, DG], f32, tag="fin")
                            nc.vector.tensor_scalar_mul(fin[:], onats[q][:], rcl[:, q : q + 1])
                            nc.sync.dma_start(out_sh[q * P : (q + 1) * P, :], fin[:])

    if not nc.is_finalized():
        nc.finalize()
    return nc


def _get_nc(reps=1):
    if reps not in _built:
        _built[reps] = _build(reps)
    return _built[reps]


def _make_in_maps(feat, sup, W_map, b_map, U, V, W_gcn):
    import ml_dtypes

    bf = ml_dtypes.bfloat16
    feat = np.ascontiguousarray(np.asarray(feat, dtype=np.float32))
    sup = np.asarray(sup, dtype=np.float32)
    W_map_f = np.asarray(W_map, dtype=np.float32)
    W_map_np = np.ascontiguousarray(W_map_f).astype(bf)
    wu_np = np.ascontiguousarray(
        W_map_f @ (SIG_A * np.asarray(U, dtype=np.float32))
    ).astype(bf)
    wv_np = np.ascontiguousarray(
        W_map_f @ (SIG_A * np.asarray(V, dtype=np.float32))
    ).astype(bf)
    b_np = np.asarray(
        SIG_A * np.asarray(b_map, dtype=np.float32).reshape(1) + SIG_B,
        dtype=np.float32,
    )
    # supT is pre-scaled by 2**100 (exact) so nonzeros >= ~6e24: this makes
    # min(sig + D, T) an exact mask-apply.  2**-100 is folded back here.
    W_gcn_np = np.ascontiguousarray(
        np.asarray(W_gcn, dtype=np.float32) * np.float32(2.0**-100)
    )

    featT = np.ascontiguousarray(feat.T).astype(bf)
    idx = np.arange(S)
    in_maps = []
    for r in range(M_CORES):
        shard = np.array(sup[r * S : (r + 1) * S, :], dtype=np.float32, copy=True)
        shard[idx, r * S + idx] += 1.0  # self loops
        shard *= np.float32(2.0**100)
        in_maps.append(
            {
                "supT": np.ascontiguousarray(shard.T).astype(bf),
                "featT": featT,
                "featTs": np.ascontiguousarray(featT[:, r * S : (r + 1) * S]),
                "W_map": W_map_np,
                "wu": wu_np,
                "wv": wv_np,
                "bfit": b_np,
                "W_gcn": W_gcn_np,
            }
        )
    return in_maps


def kernel(feat, sup, W_map, b_map, U, V, W_gcn):
    from concourse.bass_utils import run_bass_kernel_spmd

    in_maps = _make_in_maps(feat, sup, W_map, b_map, U, V, W_gcn)
    nc = _get_nc()
    trace = bool(int(os.environ.get("KERNEL_TRACE", "0")))
    try:
        res = run_bass_kernel_spmd(
            nc, in_maps, core_ids=list(range(M_CORES)), trace=trace,
            stitch_traces=False,
        )
    except Exception:
        if not trace:
            raise
        res = run_bass_kernel_spmd(
            nc, in_maps, core_ids=list(range(M_CORES)), trace=False,
            stitch_traces=False,
        )
    if trace and res.exec_time_ns is not None:
        print(f"HW exec time: {res.exec_time_ns} ns")
        kernel.last_exec_time_ns = res.exec_time_ns
        kernel.last_results = res
    out = np.concatenate(
        [res.results[r]["out_shard"] for r in range(M_CORES)], axis=0
    )
    return out.astype(np.float32)



# revision 19
# speedup vs baseline: 2.2436x; 2.2436x over previous
"""AttGNN kernel for 8 Trainium2 NeuronCores (Bass/Tile).

Math (reference):
    sup2 = sup + I
    h    = feat @ W_map                      [N, 64]
    s    = h @ U ; t = h @ V                 [N, 1]
    att  = softmax_rows(mask(tanh(s_i + t_j + b), sup2[j, i] > 0))   [N, N]
    gat  = tanh(att @ h)                     [N, 64]
    out  = normalize_rows(relu((sup2 @ gat) @ W_gcn))                [N, 64]

Distribution: 1D row-shard of sup/att over 8 cores (1024 rows each).
Both the attention mask and the two big matmuls need sup2 with the
*global* node index on SBUF partitions, i.e. the transpose of the shard
(T[c, j'] = sup2[shard j', c]).  The per-core input buffer is marshalled
host-side in that layout, pre-cast to bf16 (halves HBM traffic and
keeps the load on HWDGE instead of a gpsimd cast stream).

Softmax trick: softmax is scale-invariant, so exp(tanh(z)) can be
replaced by any g(z) with log g(z) = tanh(z) + const to within the
error budget.  g(z) = sigmoid(A z + B) + D with (A, B, D) fit by
minimax in log space matches within +/-0.32%, turning two full ACT
passes (tanh, exp) over the N x S attention block into one sigmoid
pass.  A is folded into U, V host-side; B (+ A*b_map) rides the
per-partition activation bias.

Mask + D in ONE DVE op: supT is pre-scaled by 2**100 host-side (exact,
power of two; 2**-100 is folded into W_gcn), so every nonzero entry of
T is >= 6e24 while 0 stays 0.  Then
    n = min(sigmoid(t' + s'_c) + D, T)     # scalar_tensor_tensor, 2x
equals mask * (sigmoid + D) exactly: 0 < sig + D < 1.16 << T_nonzero.
No separate mask tile, and no ucorr correction matmul stream.

Per core (c = global node index, 64 tiles of 128; j' = local shard row):
  phase 1:  unnorm[65, j'] = sum_c [h | 1][c, :]^T n[c, j']   (PE, PSUM acc)
            row 64 is the softmax denominator d[j'].
            gat[j', :] = tanh(unnorm[0:64, j'] / d[j'])
  all-gather gat (bf16) -> full [8192, 64]
  phase 2:  M[d, i'] = sum_j gat[j, d] T[j, i']               (PE)
            pre[e, i'] = W_gcn^T M                            (PE)
            out[i', :] = normalize(relu(pre))^T               -> store
"""

import os
import numpy as np

N = 8192
DIN = 128
DG = 64
M_CORES = 8
S = N // M_CORES          # 1024 shard rows per core
P = 128                   # partitions
NCT = N // P              # 64 c-tiles
NPAIR = NCT // 2          # 32 pairs of c-tiles
F2 = 2 * S                # 2048 free elems per pair tile

# minimax fit of log(sigmoid(A z + B) + D) ~ tanh(z) + const  (z in [-13, 13])
SIG_A = 2.14235191
SIG_B = -0.99688723
SIG_D = 0.15764918

_built = {}


def _build(reps=1):
    skip_tail = bool(int(os.environ.get("K_SKIP_TAIL", "0")))
    skip_p2 = bool(int(os.environ.get("K_SKIP_P2", "0")))
    skip_main = bool(int(os.environ.get("K_SKIP_MAIN", "0")))
    pool_tt = int(os.environ.get("K_POOL_TT", "8"))
    skip_ag = bool(int(os.environ.get("K_SKIP_AG", "0")))
    import concourse.bass as bass
    import concourse.bacc as bacc
    import concourse.mybir as mybir
    import concourse.tile as tile
    from concourse.masks import make_identity

    f32 = mybir.dt.float32
    bf16 = mybir.dt.bfloat16
    fp8 = mybir.dt.float8e4
    Alu = mybir.AluOpType
    Act = mybir.ActivationFunctionType

    nc = bacc.Bacc(None)

    supT = nc.declare_dram_parameter("supT", [N, S], bf16, isOutput=False)
    supT8 = nc.declare_dram_parameter("supT8", [N, S], fp8, isOutput=False)
    featT = nc.declare_dram_parameter("featT", [DIN, N], bf16, isOutput=False)
    featTs = nc.declare_dram_parameter("featTs", [DIN, S], bf16, isOutput=False)
    W_map = nc.declare_dram_parameter("W_map", [DIN, DG], bf16, isOutput=False)
    # wu = W_map @ (A U), wv = W_map @ (A V): s' = featT^T wu, t' = featTs^T wv
    wu_in = nc.declare_dram_parameter("wu", [DIN, 1], bf16, isOutput=False)
    wv_in = nc.declare_dram_parameter("wv", [DIN, 1], bf16, isOutput=False)
    bfit_in = nc.declare_dram_parameter("bfit", [1], f32, isOutput=False)
    W_gcn = nc.declare_dram_parameter("W_gcn", [DG, DG], f32, isOutput=False)
    out_sh = nc.declare_dram_parameter("out_shard", [S, DG], f32, isOutput=True)

    gat_in = nc.dram_tensor("gat_in", [S * DG], fp8)
    gat_all = nc.dram_tensor("gat_all", [M_CORES * S * DG], fp8, addr_space="Shared")

    with tile.TileContext(nc) as tc:
        with (
            tc.tile_pool(name="stat", bufs=1) as stat,
            tc.tile_pool(name="setup", bufs=2) as setup,
            tc.tile_pool(name="tpool", bufs=NPAIR) as tpool,
            tc.tile_pool(name="ring", bufs=2) as ring,
            tc.tile_pool(name="tail8", bufs=8) as tail8,
            tc.tile_pool(name="ps_acc", bufs=1, space="PSUM") as ps_acc,
            tc.tile_pool(name="psx", bufs=2, space="PSUM") as psx,
        ):
            for _rep in range(reps):
                # ---------------- constants ----------------
                ident_f = stat.tile([P, P], f32, tag="ident_f")
                make_identity(nc, ident_f[:])
                ident_b = stat.tile([P, P], bf16, tag="ident_b")
                make_identity(nc, ident_b[:])
                ones_row = stat.tile([1, P], f32, tag="ones_row")
                nc.gpsimd.memset(ones_row[:], 1.0)
                bigH = stat.tile([P, NCT * (DG + 1)], bf16, tag="bigH")
                nc.gpsimd.memset(
                    bigH[:].rearrange("p (ct w) -> p ct w", w=DG + 1)[:, :, DG : DG + 1],
                    1.0,
                )

                wmap_sb = stat.tile([DIN, DG], bf16, tag="wmap")
                nc.sync.dma_start(wmap_sb[:], W_map[:])
                wu_sb = stat.tile([DIN, 1], bf16, tag="wu")
                nc.sync.dma_start(wu_sb[:], wu_in[:])
                wv_sb = stat.tile([DIN, 1], bf16, tag="wv")
                nc.sync.dma_start(wv_sb[:], wv_in[:])
                b_sb = stat.tile([1, 1], f32, tag="b")
                nc.sync.dma_start(b_sb[:], bfit_in[:])
                wgcn_sb = stat.tile([DG, DG], f32, tag="wgcn")
                nc.sync.dma_start(wgcn_sb[:], W_gcn[:])
                # broadcast bfit to [P, 1] for per-partition activation bias
                ps_bb = psx.tile([P, 1], f32, tag="ps")
                nc.tensor.matmul(ps_bb[:], ones_row[:], b_sb[:], start=True, stop=True)
                b_bc = stat.tile([P, 1], f32, tag="b_bc")
                nc.vector.tensor_copy(b_bc[:], ps_bb[:])

                # ---------------- setup input loads (ahead of the T stream,
                # they gate t_bc / s / bigH and are small) ------------------
                fsh = setup.tile([DIN, S], bf16, tag="fch")
                nc.sync.dma_start(fsh[:], featTs[:])
                f_chunks = []
                for g in range(8):
                    fch = setup.tile([DIN, S], bf16, tag=f"fg{g}", bufs=1)
                    f_chunks.append(fch)
                    nc.sync.dma_start(fch[:], featT[:, g * S : (g + 1) * S])

                # ---------------- T load streams ---------------------------
                # bf16 T' (mask-min path): ring, consumed in phase 1 only.
                t_tiles = []
                for p in range(NPAIR):
                    tp = tpool.tile([P, F2], bf16, tag="T", bufs=6)
                    t_tiles.append(tp)
                    for half in range(2):
                        nc.sync.dma_start(
                            tp[:, half * S : (half + 1) * S],
                            supT[p * 256 + half * P : p * 256 + (half + 1) * P, :],
                        )
                # fp8 T8 (phase-2 aggregation values, x64 scale): resident.
                # Queued behind T' on the SP queue (T' paces phase 1); the
                # tail's gat store/reload go on the Activation queue instead.
                t8_tiles = []
                for p in range(NPAIR):
                    tp8 = tpool.tile([P, F2], fp8, tag=f"T8_{p}", bufs=1)
                    t8_tiles.append(tp8)
                    for half in range(2):
                        nc.sync.dma_start(
                            tp8[:, half * S : (half + 1) * S],
                            supT8[p * 256 + half * P : p * 256 + (half + 1) * P, :],
                        )

                # ---------------- setup: t first, then s / bigH ----------------
                # t'_bc[p, j] = (W_map V')^T featTs = wv^T featTs, already
                # broadcast to all 128 partitions: lhsT = wv replicated.
                wvB = stat.tile([DIN, P], bf16, tag="wvB")
                nc.vector.tensor_copy(wvB[:], wv_sb[:].broadcast_to([DIN, P]))
                t_bc = stat.tile([P, S], bf16, tag="t_bc")
                for half in range(2):
                    ps_b = psx.tile([P, 512], f32, tag="ps")
                    nc.tensor.matmul(
                        ps_b[:], wvB[:], fsh[:, half * 512 : (half + 1) * 512],
                        start=True, stop=True,
                    )
                    nc.vector.tensor_copy(t_bc[:, half * 512 : (half + 1) * 512], ps_b[:])

                # s' and h-tiles, chunk by chunk; s' in 8 small tiles so the
                # main loop can start as soon as the first chunk is done.
                sb_tiles = []
                for g in range(8):
                    fch = f_chunks[g]
                    # s' column per c-tile: featT_chunk^T @ wu
                    ps_s = psx.tile([P, 8], f32, tag="ps")
                    for k in range(8):
                        nc.tensor.matmul(
                            ps_s[:, k : k + 1],
                            fch[:, k * P : (k + 1) * P],
                            wu_sb[:],
                            start=True,
                            stop=True,
                        )
                    # s_b = s' + (A*b_map + B): per-partition sigmoid bias
                    s_g = stat.tile([P, 8], f32, tag=f"s_{g}")
                    sb_tiles.append(s_g)
                    nc.scalar.activation(
                        s_g[:], ps_s[:], Act.Identity, bias=b_bc[:], scale=1.0
                    )
                    # bigH h-chunks: batch the 8 PSUM->SBUF copies into one
                    ps_bh = psx.tile([P, 8 * DG], f32, tag="ps")
                    for k in range(8):
                        nc.tensor.matmul(
                            ps_bh[:, k * DG : (k + 1) * DG],
                            fch[:, k * P : (k + 1) * P],
                            wmap_sb[:],
                            start=True,
                            stop=True,
                        )
                    nc.vector.tensor_copy(
                        bigH[:, g * 8 * (DG + 1) : (g + 1) * 8 * (DG + 1)]
                        .rearrange("p (ct w) -> p ct w", w=DG + 1)[:, :, 0:DG],
                        ps_bh[:].rearrange("p (ct w) -> p ct w", w=DG),
                    )

                # PSUM accumulators (1 bank each)
                un0 = ps_acc.tile([DG + 1, 512], f32, tag="un0")
                un1 = ps_acc.tile([DG + 1, 512], f32, tag="un1")
                unnorm = (un0, un1)
                m0 = ps_acc.tile([DG, 512], f32, tag="m0")
                m1 = ps_acc.tile([DG, 512], f32, tag="m1")
                mm = (m0, m1)

                # ---------------- phase 1 main loop ----------------
                for p in ([] if skip_main else range(NPAIR)):
                    tp = t_tiles[p]
                    g = ring.tile([P, F2], bf16, tag="g")
                    for half in range(2):
                        ct = 2 * p + half
                        nc.scalar.activation(
                            g[:, half * S : (half + 1) * S], t_bc[:],
                            Act.Sigmoid, bias=sb_tiles[ct // 8][:, ct % 8 : ct % 8 + 1],
                            scale=1.0,
                        )
                    # g2 = g + D (tensor_scalar, 4x, out-of-place so ACT can
                    # reuse g's buffer early); n = min(g2, T) (tt, 2x) —
                    # the scaled-T trick makes min() the whole mask apply.
                    g2 = ring.tile([P, F2], bf16, tag="g2")
                    nc.vector.tensor_scalar_add(g2[:], g[:], SIG_D)
                    n = ring.tile([P, F2], bf16, tag="n")
                    nc.vector.tensor_tensor(n[:], g2[:], tp[:], Alu.min)
                    for half in range(2):
                        ct = 2 * p + half
                        lhs = bigH[:, ct * (DG + 1) : (ct + 1) * (DG + 1)]
                        for jb in range(2):
                            sl = slice(half * S + jb * 512, half * S + (jb + 1) * 512)
                            nc.tensor.matmul(
                                unnorm[jb][:], lhs, n[:, sl],
                                start=(p == 0 and half == 0),
                                stop=(p == NPAIR - 1 and half == 1),
                            )

                if skip_tail:
                    zz = tail8.tile([P, DG], f32, tag='fin')
                    nc.vector.memset(zz[:], 0.0)
                    for q in range(8):
                        nc.sync.dma_start(out_sh[q * P : (q + 1) * P, :], zz[:])
                else:
                    # ---------------- tail: gat, all-gather -----------------------
                    # unnorm -> SBUF (ACT, free post-phase-1), transpose each
                    # 128-column block to [j, 65] (PE), then gat = tanh(un/d)
                    # lands DIRECTLY in the fp8 gn tile via ACT's
                    # per-partition scale port (scale = 1/d[j]).
                    unsb = ring.tile([DG + 1, S], f32, tag="g")
                    for jb in range(2):
                        nc.scalar.activation(
                            unsb[:, jb * 512 : (jb + 1) * 512],
                            unnorm[jb][:], Act.Copy,
                        )
                    gn = stat.tile([P, 8 * DG], fp8, tag="gn")
                    rsc = stat.tile([P, 8], f32, tag="rsc")
                    for q in range(8):
                        ps_g = psx.tile([P, DG + 1], f32, tag="ps")
                        nc.tensor.transpose(
                            ps_g[:], unsb[:, q * P : (q + 1) * P],
                            ident_f[0 : DG + 1, 0 : DG + 1],
                        )
                        nc.vector.reciprocal(
                            rsc[:, q : q + 1], ps_g[:, DG : DG + 1]
                        )
                        nc.scalar.activation(
                            gn[:, q * DG : (q + 1) * DG], ps_g[:, 0:DG],
                            Act.Tanh, scale=rsc[:, q : q + 1],
                        )
                    nc.scalar.dma_start(
                        gat_in[:].rearrange("(q p d) -> p q d", q=8, p=P),
                        gn[:].rearrange("p (q d) -> p q d", d=DG),
                    )
                    # preload the sqrt act-table set under the collective so
                    # the final normalize doesn't eat a LoadActFuncSet.
                    sq_dummy = stat.tile([1, 1], f32, tag="sqd")
                    nc.scalar.activation(sq_dummy[:], b_bc[0:1, :], Act.Sqrt)
                    if not skip_ag:
                        nc.gpsimd.collective_compute(
                            "AllGather",
                            Alu.bypass,
                            replica_groups=[list(range(M_CORES))],
                            ins=[gat_in[:]],
                            outs=[gat_all[:]],
                        )
                    else:
                        nc.sync.dma_start(gat_all[0 : S * DG], gat_in[:])
                    gat_sb = []
                    for hh in range(2):
                        # reuse a dead featT-chunk buffer (same byte size)
                        gsb8 = setup.tile([P, 32 * DG], fp8, tag=f"fg{hh}", bufs=1)
                        gat_sb.append(gsb8)
                        nc.scalar.dma_start(
                            gsb8[:].rearrange("p (ct d) -> p ct d", d=DG),
                            gat_all[
                                hh * 32 * P * DG : (hh + 1) * 32 * P * DG
                            ].rearrange("(ct p d) -> p ct d", p=P, d=DG),
                        )

                    if skip_p2:
                        zz2 = tail8.tile([P, DG], f32, tag='fin')
                        nc.vector.memset(zz2[:], 0.0)
                        for q in range(8):
                            nc.sync.dma_start(out_sh[q * P : (q + 1) * P, :], zz2[:])
                    else:
                        # ---------------- phase 2 (jb-outer: jb=0's post-chain
                        # overlaps jb=1's matmul sweep) ----------------
                        m_sb = ring.tile([DG, S], f32, tag="mc")
                        reluT = ring.tile([DG, S], f32, tag="n")
                        onats = [None] * 8
                        n2_all = stat.tile([P, 8], f32, tag="n2_all")
                        sqs = stat.tile([P, DG], f32, tag="sqs")
                        for jb in range(2):
                            first = True
                            for hh in range(2):
                                for q in range(16):
                                    # ctile pair (hh*32+2q, hh*32+2q+1), both
                                    # contracted in ONE fp8 DoubleRow matmul.
                                    p_idx = hh * 16 + q
                                    lhs = gat_sb[hh][
                                        :, (2 * q) * DG : (2 * q + 2) * DG
                                    ].rearrange("p (two d) -> p two d", two=2)
                                    rhs = (
                                        t8_tiles[p_idx][:]
                                        .rearrange("p (two s) -> p two s", two=2)
                                        [:, :, jb * 512 : (jb + 1) * 512]
                                    )
                                    nc.tensor.matmul(
                                        mm[jb][:],
                                        lhs,
                                        rhs,
                                        start=first,
                                        stop=(hh == 1 and q == 15),
                                        perf_mode=mybir.MatmulPerfMode.DoubleRow,
                                    )
                                    first = False
                            nc.vector.tensor_copy(
                                m_sb[:, jb * 512 : (jb + 1) * 512], mm[jb][:]
                            )
                            ps_o = psx.tile([DG, 512], f32, tag="ps")
                            nc.tensor.matmul(
                                ps_o[:], wgcn_sb[:], m_sb[:, jb * 512 : (jb + 1) * 512],
                                start=True, stop=True,
                            )
                            nc.scalar.activation(
                                reluT[:, jb * 512 : (jb + 1) * 512], ps_o[:], Act.Relu
                            )
                            for q in range(jb * 4, jb * 4 + 4):
                                ps_t2 = psx.tile([P, DG], f32, tag="ps")
                                nc.tensor.transpose(
                                    ps_t2[:], reluT[:, q * P : (q + 1) * P],
                                    ident_f[0:DG, 0:DG],
                                )
                                onat = tail8.tile([P, DG], f32, tag="onat")
                                nc.vector.tensor_copy(onat[:], ps_t2[:])
                                onats[q] = onat
                                nc.scalar.activation(
                                    sqs[:], ps_t2[:], Act.Square,
                                    accum_out=n2_all[:, q : q + 1],
                                )

                        # ---------------- normalize + store ----------------
                        nrm = stat.tile([P, 8], f32, tag="nrm")
                        nc.scalar.activation(nrm[:], n2_all[:], Act.Sqrt)
                        nc.vector.tensor_scalar_max(nrm[:], nrm[:], 1e-12)
                        rcl = stat.tile([P, 8], f32, tag="rcl")
                        nc.vector.reciprocal(rcl[:], nrm[:])
                        for q in range(8):
                            fin = tail8.tile([P, DG], f32, tag="fin")
                            nc.vector.tensor_scalar_mul(fin[:], onats[q][:], rcl[:, q : q + 1])
                            nc.sync.dma_start(out_sh[q * P : (q + 1) * P, :], fin[:])

    if not nc.is_finalized():
        nc.finalize()
    return nc


def _get_nc(reps=1):
    if reps not in _built:
        _built[reps] = _build(reps)
    return _built[reps]


def _make_in_maps(feat, sup, W_map, b_map, U, V, W_gcn):
    import ml_dtypes

    bf = ml_dtypes.bfloat16
    feat = np.ascontiguousarray(np.asarray(feat, dtype=np.float32))
    sup = np.asarray(sup, dtype=np.float32)
    W_map_f = np.asarray(W_map, dtype=np.float32)
    W_map_np = np.ascontiguousarray(W_map_f).astype(bf)
    wu_np = np.ascontiguousarray(
        W_map_f @ (SIG_A * np.asarray(U, dtype=np.float32))
    ).astype(bf)
    wv_np = np.ascontiguousarray(
        W_map_f @ (SIG_A * np.asarray(V, dtype=np.float32))
    ).astype(bf)
    b_np = np.asarray(
        SIG_A * np.asarray(b_map, dtype=np.float32).reshape(1) + SIG_B,
        dtype=np.float32,
    )
    # supT (bf16) is pre-scaled by 2**100 (exact) so nonzeros >= ~6e24: this
    # makes min(sig + D, T) an exact mask-apply; it feeds NO value path.
    # supT8 (fp8 e4m3) carries the actual sup2 values for phase 2, scaled by
    # 64 (exact; max |sup2| < 2 -> <= 128 < 240 fp8 max); 1/64 is folded into
    # W_gcn here.
    W_gcn_np = np.ascontiguousarray(
        np.asarray(W_gcn, dtype=np.float32) * np.float32(1.0 / 64.0)
    )
    import ml_dtypes as _mld

    f8 = _mld.float8_e4m3

    featT = np.ascontiguousarray(feat.T).astype(bf)
    idx = np.arange(S)
    in_maps = []
    for r in range(M_CORES):
        shard = np.array(sup[r * S : (r + 1) * S, :], dtype=np.float32, copy=True)
        shard[idx, r * S + idx] += 1.0  # self loops
        shardT = np.ascontiguousarray(shard.T)
        in_maps.append(
            {
                "supT": (shardT * np.float32(2.0**100)).astype(bf),
                "supT8": (shardT * np.float32(64.0)).astype(f8),
                "featT": featT,
                "featTs": np.ascontiguousarray(featT[:, r * S : (r + 1) * S]),
                "W_map": W_map_np,
                "wu": wu_np,
                "wv": wv_np,
                "bfit": b_np,
                "W_gcn": W_gcn_np,
            }
        )
    return in_maps


def kernel(feat, sup, W_map, b_map, U, V, W_gcn):
    from concourse.bass_utils import run_bass_kernel_spmd

    in_maps = _make_in_maps(feat, sup, W_map, b_map, U, V, W_gcn)
    nc = _get_nc()
    trace = bool(int(os.environ.get("KERNEL_TRACE", "0")))
    try:
        res = run_bass_kernel_spmd(
            nc, in_maps, core_ids=list(range(M_CORES)), trace=trace,
            stitch_traces=False,
        )
    except Exception:
        if not trace:
            raise
        res = run_bass_kernel_spmd(
            nc, in_maps, core_ids=list(range(M_CORES)), trace=False,
            stitch_traces=False,
        )
    if trace and res.exec_time_ns is not None:
        print(f"HW exec time: {res.exec_time_ns} ns")
        kernel.last_exec_time_ns = res.exec_time_ns
        kernel.last_results = res
    out = np.concatenate(
        [res.results[r]["out_shard"] for r in range(M_CORES)], axis=0
    )
    return out.astype(np.float32)

